# revision 4
# baseline (speedup 1.0000x reference)
"""DSAttention layer for Trainium2, 8 NeuronCores.

Sharding: core c -> batch b = c//2, head-group g = c%2 (4 heads each,
e-columns 256g..256g+255 of the 512-wide head dim).  tau[b]/8 (softmax
temperature x 1/sqrt(E)) is folded into each core's Wq/bq slice on the
host; delta[b] broadcasts over the softmax axis and is shift-invariant,
so it drops out exactly.  Each core emits its head-group's partial
output projection [2048, 512] fp16; the host sums the pair per batch
in fp32 and adds (bv @ Wo + bo).

Host pre-transposes X^T (d-major, fp16) so the device does ZERO
PE transposes and no f32->f16 downcasts.

Device dataflow per core (all matmul operands fp16, fp32 PSUM accum):
  qT/kT [e 256, l 2048] = W^T @ X^T   (e on partitions)
  v_aug [s 2048, 128] per (s-tile, head): cols 0-63 = V, cols 64-127
        all-ones -> AV matmul rows 64-127 all equal the softmax
        denominator Z (no separate Z machinery).
  scoresT[s,l] = kT.T @ qT  per head, head pairs concurrent via
        partition-offset row groups (K=64 at rows 0-63/64-127),
        written into wide PSUM groups of 4 (A) / 2 (B) [128,512] units
  E = exp(scoresT - 2) fp16  (ONE ACT instr per group: amortizes the
        ~352-cycle ACT fixed cost; A/B alternation = double buffer
        within the 8-bank PSUM budget: 4 + 2 + 2 AV accumulators)
  av[128, l 512] = v_aug.T @ E  (accumulate 16 s-chunks in PSUM;
        rows 64-127 = Z) -> attnT[hh*64:, ec, l] = av[0:64] * (1/Z)
        (DVE reciprocal of av[64:128] + one tensor-tensor mul)
  out[l,512] = attnT.T @ Wo  (K=128: head pairs packed on partitions)
"""

import numpy as np
from contextlib import ExitStack

import concourse.bass as bass
import concourse.bacc as bacc
import concourse.mybir as mybir
import concourse.tile as tile
from concourse.bass_utils import run_bass_kernel_spmd

F32 = mybir.dt.float32
F16 = mybir.dt.float16

B, L, S, D = 4, 2048, 2048, 512
H, E = 8, 64          # full model heads / head dim
HG = 4                # heads per core (head-group)
EG = HG * E           # 256, e-columns per core
N_CORES = 8

ST = S // 128         # 16 s-tiles
DC = D // 128         # 4 d-chunks
LQ = 4                # l-quarters of 512
SCALE = 1.0 / np.sqrt(np.float32(E))
EXP_SHIFT = -2.0      # exp(x-2): cancels in softmax, guards fp16 overflow


def _emit(ctx: ExitStack, tc: "tile.TileContext", io: dict):
    nc = tc.nc
    mm = nc.tensor.matmul

    singles = ctx.enter_context(tc.tile_pool(name="singles", bufs=1))
    bigs = ctx.enter_context(tc.tile_pool(name="bigs", bufs=1))
    xt_pool = ctx.enter_context(tc.tile_pool(name="xt", bufs=2))
    e_pool = ctx.enter_context(tc.tile_pool(name="eslab", bufs=2))
    rz_pool = ctx.enter_context(tc.tile_pool(name="rz", bufs=2))
    at_pool = ctx.enter_context(tc.tile_pool(name="attnT", bufs=2))
    ob_pool = ctx.enter_context(tc.tile_pool(name="outsb", bufs=3))

    # ---- constants & weights -------------------------------------------
    shift_col = singles.tile([128, 1], F32)
    nc.vector.memset(shift_col, EXP_SHIFT)
    warm = singles.tile([128, 1], F16)

    wq_sb = singles.tile([128, DC, EG], F16)   # [p, c, e] = Wq[c*128+p, e]
    wk_sb = singles.tile([128, DC, EG], F16)
    wv_sb = singles.tile([128, DC, EG], F16)
    wo_sb = singles.tile([128, 2, D], F16)     # [p, g2, n] = Wo[128*g2+p, n]
    bq_sb = singles.tile([128, 2], F32)        # [p, ec] = bq[128ec+p]
    bk_sb = singles.tile([128, 2], F32)
    nc.sync.dma_start(out=wq_sb, in_=io["wq"][:])
    nc.sync.dma_start(out=wk_sb, in_=io["wk"][:])
    nc.sync.dma_start(out=wv_sb, in_=io["wv"][:])
    nc.sync.dma_start(out=wo_sb, in_=io["wo"][:])
    nc.sync.dma_start(out=bq_sb, in_=io["bq"][:])
    nc.sync.dma_start(out=bk_sb, in_=io["bk"][:])

    # warm the ACT exp table-set (~2.7us) during the prolog
    nc.scalar.activation(out=warm, in_=shift_col,
                         func=mybir.ActivationFunctionType.Exp,
                         bias=shift_col[:, 0:1], scale=1.0)

    # ---- big persistent SBUF tensors -----------------------------------
    qT = bigs.tile([128, 2, L], F16, tag="qT")     # [e_in_chunk, ec, l]
    kT = bigs.tile([128, 2, S], F16, tag="kT")
    v_sb = bigs.tile([128, ST, HG, 128], F16, tag="v")  # [s_in_tile, st, h, 0:64 v | 64:128 ones]
    nc.vector.memset(v_sb[:, :, :, 64:128], 1.0)  # ones cols -> Z rows

    # ---- projections (host-provided X^T fp16, no transposes) -----------
    def load_xt(x_dram, lc):
        xt = xt_pool.tile([128, DC, 512], F16, tag="xt")
        nc.sync.dma_start(out=xt, in_=x_dram[:, :, lc * 512:(lc + 1) * 512])
        return xt

    def proj_qk(xt, w_sb, b_sb, dst, lc, pj_pool):
        # dst[:, ec, 512lc : 512lc+512] = (W.T @ X^T) + bias
        for ec in range(2):
            pp = pj_pool.tile([128, 512], F32, tag="pj", name=f"pp_{lc}_{ec}")
            for c in range(DC):
                mm(pp, lhsT=w_sb[:, c, ec * 128:(ec + 1) * 128],
                   rhs=xt[:, c, :], start=(c == 0), stop=(c == DC - 1))
            nc.vector.tensor_scalar_add(
                out=dst[:, ec, lc * 512:(lc + 1) * 512], in0=pp,
                scalar1=b_sb[:, ec:ec + 1])

    def proj_v(xt, lc, pj_pool):
        for i in range(4):
            st = lc * 4 + i
            vp = pj_pool.tile([128, 512], F32, tag="pj",
                              name=f"vp_{lc}_{i}")[:, 0:EG]
            for c in range(DC):
                mm(vp, lhsT=xt[:, c, i * 128:(i + 1) * 128],
                   rhs=wv_sb[:, c, :], start=(c == 0), stop=(c == DC - 1))
            nc.vector.tensor_copy(
                out=v_sb[:, st, :, 0:64],
                in_=vp.rearrange("p (h e) -> p h e", h=HG))

    with tc.tile_pool(name="ps_pj", bufs=2, space="PSUM") as pj_pool:
        for lc in range(4):
            xt = load_xt(io["xk"], lc)
            proj_qk(xt, wk_sb, bk_sb, kT, lc, pj_pool)
        for lc in range(4):
            xt = load_xt(io["xv"], lc)
            proj_v(xt, lc, pj_pool)
        for lc in range(4):
            xt = load_xt(io["xq"], lc)
            proj_qk(xt, wq_sb, bq_sb, qT, lc, pj_pool)

    # ---- attention ------------------------------------------------------
    # PSUM budget (8 banks): scA 4 + scB 2 + av/op 2.
    ps_scA = ctx.enter_context(tc.tile_pool(name="ps_scA", bufs=1, space="PSUM"))
    ps_scB = ctx.enter_context(tc.tile_pool(name="ps_scB", bufs=1, space="PSUM"))
    ps_av = ctx.enter_context(tc.tile_pool(name="ps_av", bufs=2, space="PSUM"))

    # unit u = (j, hh): j = u//2 (s-tile), hh = u%2 (head in pair).
    # groups alternate A(4 units)/B(2 units): A is j,j+1; B is j+2.
    groups = []
    u = 0
    while u < 32:
        n = 4 if (len(groups) % 2 == 0) else 2
        n = min(n, 32 - u)
        groups.append((u, n))
        u += n

    for lq in range(LQ):
        l0 = lq * 512
        attnT = at_pool.tile([128, 2, 512], F16, tag="at")
        for ec in range(2):
            av = [ps_av.tile([128, 512], F32, tag="av",
                             name=f"av{lq}_{ec}_{i}") for i in range(2)]
            for gi, (u0, n) in enumerate(groups):
                a_side = (gi % 2 == 0)
                pool = ps_scA if a_side else ps_scB
                full = pool.tile([128, 4 if a_side else 2, 512], F32,
                                 tag="sc", name=f"sc{lq}_{ec}_{gi}")
                sc = full[:, 0:n, :]
                for k in range(n):
                    j, hh = (u0 + k) // 2, (u0 + k) % 2
                    o = hh * 64
                    mm(sc[:, k, :],
                       lhsT=kT[o:o + 64, ec, j * 128:(j + 1) * 128],
                       rhs=qT[o:o + 64, ec, l0:l0 + 512],
                       start=True, stop=True, tile_position=(o, 0))
                ep_full = e_pool.tile([128, 4, 512], F16, tag="ep",
                                      name=f"ep{lq}_{ec}_{gi}")
                ep = ep_full[:, 0:n, :]
                nc.scalar.activation(out=ep, in_=sc,
                                     func=mybir.ActivationFunctionType.Exp,
                                     bias=shift_col[:, 0:1], scale=1.0)
                for k in range(n):
                    j, hh = (u0 + k) // 2, (u0 + k) % 2
                    mm(av[hh], lhsT=v_sb[:, j, 2 * ec + hh, :],
                       rhs=ep[:, k, :], start=(j == 0), stop=(j == ST - 1))
            for hh in range(2):
                rz = rz_pool.tile([64, 512], F16, tag="rz")
                with nc.allow_low_precision(reason="1/Z in f16"):
                    nc.vector.reciprocal(rz, av[hh][64:128, :])
                nc.vector.tensor_mul(
                    out=attnT[hh * 64:hh * 64 + 64, ec, :],
                    in0=av[hh][0:64, :], in1=rz)
        # output projection for this l-quarter (K=128: packed head pairs)
        for i in range(4):
            lt = lq * 4 + i
            op = ps_av.tile([128, D], F32, tag="av", name=f"op_{lq}_{i}")
            for g2 in range(2):
                mm(op, lhsT=attnT[:, g2, i * 128:(i + 1) * 128],
                   rhs=wo_sb[:, g2, :], start=(g2 == 0), stop=(g2 == 1))
            ob = ob_pool.tile([128, D], F16, tag="ob")
            nc.vector.tensor_copy(out=ob, in_=op)
            nc.sync.dma_start(out=io["out"][lt * 128:(lt + 1) * 128, :], in_=ob)


def build_nc():
    nc = bacc.Bacc()
    io = {}
    io["xq"] = nc.declare_dram_parameter("xq", [128, DC, L], F16, isOutput=False)
    io["xk"] = nc.declare_dram_parameter("xk", [128, DC, S], F16, isOutput=False)
    io["xv"] = nc.declare_dram_parameter("xv", [128, DC, S], F16, isOutput=False)
    io["wq"] = nc.declare_dram_parameter("wq", [128, DC, EG], F16, isOutput=False)
    io["wk"] = nc.declare_dram_parameter("wk", [128, DC, EG], F16, isOutput=False)
    io["wv"] = nc.declare_dram_parameter("wv", [128, DC, EG], F16, isOutput=False)
    io["wo"] = nc.declare_dram_parameter("wo", [128, 2, D], F16, isOutput=False)
    io["bq"] = nc.declare_dram_parameter("bq", [128, 2], F32, isOutput=False)
    io["bk"] = nc.declare_dram_parameter("bk", [128, 2], F32, isOutput=False)
    io["out"] = nc.declare_dram_parameter("out", [L, D], F16, isOutput=True)
    with tile.TileContext(nc) as tc:
        with ExitStack() as ctx:
            _emit(ctx, tc, io)
    nc.compile()
    return nc


_NC = None


def _get_nc():
    global _NC
    if _NC is None:
        _NC = build_nc()
    return _NC


def _chunk_w(w):
    """[512, n] -> [128, 4, n] fp16:  [p, c, :] = w[128c+p, :]"""
    n = w.shape[1]
    return np.ascontiguousarray(
        w.reshape(DC, 128, n).transpose(1, 0, 2), dtype=np.float16)


def _chunk_xt(x):
    """[2048, 512] -> X^T chunked [128, 4, 2048] fp16: [p, c, l] = x[l, 128c+p]"""
    return np.ascontiguousarray(
        np.asarray(x, dtype=np.float16).T.reshape(DC, 128, L).transpose(1, 0, 2))


def make_in_maps(queries, keys, values, tau, Wq, bq, Wk, bk, Wv, bv, Wo):
    xts = [{"xq": _chunk_xt(queries[b]), "xk": _chunk_xt(keys[b]),
            "xv": _chunk_xt(values[b])} for b in range(B)]
    in_maps = []
    for c in range(N_CORES):
        b, g = c // 2, c % 2
        e0 = g * EG
        f = np.float32(SCALE * tau[b])
        wq = _chunk_w(Wq[:, e0:e0 + EG] * f)
        wk = _chunk_w(Wk[:, e0:e0 + EG])
        wv = _chunk_w(Wv[:, e0:e0 + EG])
        wo = np.ascontiguousarray(
            Wo[e0:e0 + EG, :].reshape(2, 128, D).transpose(1, 0, 2),
            dtype=np.float16)
        in_maps.append({
            **xts[b],
            "wq": wq, "wk": wk, "wv": wv, "wo": wo,
            "bq": np.ascontiguousarray(
                (bq[e0:e0 + EG] * f).reshape(2, 128).T, dtype=np.float32),
            "bk": np.ascontiguousarray(
                bk[e0:e0 + EG].reshape(2, 128).T, dtype=np.float32),
        })
    return in_maps


def kernel(queries, keys, values, tau, delta, Wq, bq, Wk, bk, Wv, bv, Wo, bo,
           **_unused):
    queries = np.asarray(queries, dtype=np.float32)
    keys = np.asarray(keys, dtype=np.float32)
    values = np.asarray(values, dtype=np.float32)
    tau = np.asarray(tau, dtype=np.float32)
    Wq, bq = np.asarray(Wq, np.float32), np.asarray(bq, np.float32)
    Wk, bk = np.asarray(Wk, np.float32), np.asarray(bk, np.float32)
    Wv, bv = np.asarray(Wv, np.float32), np.asarray(bv, np.float32)
    Wo, bo = np.asarray(Wo, np.float32), np.asarray(bo, np.float32)

    nc = _get_nc()
    in_maps = make_in_maps(queries, keys, values, tau, Wq, bq, Wk, bk, Wv, bv, Wo)
    res = run_bass_kernel_spmd(nc, in_maps, list(range(N_CORES)))
    # attn rows sum to 1 -> +bv flows through Wo as a constant row; + bo.
    const_row = (bv @ Wo + bo).astype(np.float32)  # [512]
    out = np.empty((B, L, D), dtype=np.float32)
    for b in range(B):
        out[b] = res.results[2 * b]["out"].astype(np.float32) \
            + res.results[2 * b + 1]["out"].astype(np.float32) + const_row
    return out


if __name__ == "__main__":
    nc = build_nc()
    print("built OK")


# revision 7
# speedup vs baseline: 1.0513x; 1.0513x over previous
"""DSAttention layer for Trainium2, 8 NeuronCores.

Sharding: core c -> batch b = c//2, head-group g = c%2 (4 heads each,
e-columns 256g..256g+255 of the 512-wide head dim).  tau[b]/8 (softmax
temperature x 1/sqrt(E)) is folded into each core's Wq/bq slice on the
host; delta[b] broadcasts over the softmax axis and is shift-invariant,
so it drops out exactly.  Each core emits its head-group's partial
output projection [2048, 512] fp16; the host sums the pair per batch
in fp32 and adds (bv @ Wo + bo).

Host pre-transposes X^T (d-major, fp16) so the device does ZERO
PE transposes and no f32->f16 downcasts.

Device dataflow per core (all matmul operands fp16, fp32 PSUM accum):
  qT/kT [e 256, l 2048] = W^T @ X^T   (e on partitions)
  v_aug [s 2048, 128] per (s-tile, head): cols 0-63 = V, cols 64-127
        all-ones -> AV matmul rows 64-127 all equal the softmax
        denominator Z (no separate Z machinery).
  scoresT[s,l] = kT.T @ qT  per head, head pairs on partition-offset
        row groups (K=64 at rows 0-63/64-127), written into wide PSUM
        groups of 4 (A) / 2 (B) [128,512] units
  E = exp(scoresT - 2) fp16  (ONE ACT instr per group: amortizes the
        ~352-cycle ACT fixed cost; A/B alternation = double buffer
        within the 8-bank PSUM budget: 4 + 2 + 2 AV accumulators)
  av[128, l 512] = v_aug.T @ E  (accumulate 16 s-chunks in PSUM;
        rows 64-127 = Z) -> attnT[hh*64:, ec, l] = av[0:64] * (1/Z)
        (DVE reciprocal_approx_fast + one tensor-tensor mul)
  out[l,512] = attnT.T @ Wo  (K=128: head pairs packed on partitions)

Scheduling: normalize / output-projection / next q-projection are
DEFERRED into the next block, emitted right after its first score
group, so this PE + DVE work lands inside the PE's exp-wait gap
instead of stalling the block boundary.
"""

import numpy as np
from contextlib import ExitStack

import concourse.bass as bass
import concourse.bacc as bacc
import concourse.mybir as mybir
import concourse.tile as tile
from concourse.bass_utils import run_bass_kernel_spmd

F32 = mybir.dt.float32
F16 = mybir.dt.float16

B, L, S, D = 4, 2048, 2048, 512
H, E = 8, 64          # full model heads / head dim
HG = 4                # heads per core (head-group)
EG = HG * E           # 256, e-columns per core
N_CORES = 8

ST = S // 128         # 16 s-tiles
DC = D // 128         # 4 d-chunks
LQ = 4                # l-quarters of 512
SCALE = 1.0 / np.sqrt(np.float32(E))
EXP_SHIFT = -2.0      # exp(x-2): cancels in softmax, guards fp16 overflow

# unit u = (j, hh): j = u//2 (s-tile), hh = u%2 (head in pair).
# groups alternate scA(4 units)/scB(2 units); last group rides scA.
GROUPS = []
_u = 0
while _u < 32:
    _n = min(4 if (len(GROUPS) % 2 == 0) else 2, 32 - _u)
    GROUPS.append((_u, _n))
    _u += _n


def _emit(ctx: ExitStack, tc: "tile.TileContext", io: dict):
    nc = tc.nc
    mm = nc.tensor.matmul

    singles = ctx.enter_context(tc.tile_pool(name="singles", bufs=1))
    bigs = ctx.enter_context(tc.tile_pool(name="bigs", bufs=1))
    xt_pool = ctx.enter_context(tc.tile_pool(name="xt", bufs=2))
    xq_pool = ctx.enter_context(tc.tile_pool(name="xq", bufs=2))
    e_pool = ctx.enter_context(tc.tile_pool(name="eslab", bufs=2))
    rz_pool = ctx.enter_context(tc.tile_pool(name="rz", bufs=4))
    at_pool = ctx.enter_context(tc.tile_pool(name="attnT", bufs=2))
    ob_pool = ctx.enter_context(tc.tile_pool(name="outsb", bufs=3))

    # ---- constants & first weights -------------------------------------
    shift_col = singles.tile([128, 1], F32)
    nc.vector.memset(shift_col, EXP_SHIFT)
    warm = singles.tile([128, 1], F16)

    wq_sb = singles.tile([128, DC, EG], F16)   # [p, c, e] = Wq[c*128+p, e]
    wk_sb = singles.tile([128, DC, EG], F16)
    wv_sb = singles.tile([128, DC, EG], F16)
    wo_sb = singles.tile([128, 2, D], F16)     # [p, g2, n] = Wo[128*g2+p, n]
    bq_sb = singles.tile([128, 2], F32)        # [p, ec] = bq[128ec+p]
    bk_sb = singles.tile([128, 2], F32)
    # k path first so the first matmul starts ASAP
    nc.sync.dma_start(out=wk_sb, in_=io["wk"][:])
    nc.sync.dma_start(out=bk_sb, in_=io["bk"][:])

    # warm the ACT exp table-set (~2.7us) during the prolog
    nc.scalar.activation(out=warm, in_=shift_col,
                         func=mybir.ActivationFunctionType.Exp,
                         bias=shift_col[:, 0:1], scale=1.0)

    # ---- big persistent SBUF tensors -----------------------------------
    qT = bigs.tile([128, 2, L], F16, tag="qT")     # [e_in_chunk, ec, l]
    kT = bigs.tile([128, 2, S], F16, tag="kT")
    v_sb = bigs.tile([128, ST, HG, 128], F16, tag="v")  # [.., 0:64 v | 64:128 ones]
    nc.vector.memset(v_sb[:, :, :, 64:128], 1.0)  # ones cols -> Z rows

    # ---- projections (host-provided X^T fp16, no transposes) -----------
    def load_xt(pool, x_dram, lc, nm):
        xt = pool.tile([128, DC, 512], F16, tag="xt", name=nm)
        nc.sync.dma_start(out=xt, in_=x_dram[:, :, lc * 512:(lc + 1) * 512])
        return xt

    def proj_qk(xt, w_sb, b_sb, dst, lc, pj_pool, tag):
        # dst[:, ec, 512lc : 512lc+512] = (W.T @ X^T) + bias
        for ec in range(2):
            pp = pj_pool.tile([128, 512], F32, tag=tag, name=f"pp_{lc}_{ec}")
            for c in range(DC):
                mm(pp, lhsT=w_sb[:, c, ec * 128:(ec + 1) * 128],
                   rhs=xt[:, c, :], start=(c == 0), stop=(c == DC - 1))
            nc.vector.tensor_scalar_add(
                out=dst[:, ec, lc * 512:(lc + 1) * 512], in0=pp,
                scalar1=b_sb[:, ec:ec + 1])

    def proj_v(xt, lc, pj_pool):
        for i in range(4):
            st = lc * 4 + i
            vp = pj_pool.tile([128, 512], F32, tag="pj",
                              name=f"vp_{lc}_{i}")[:, 0:EG]
            for c in range(DC):
                mm(vp, lhsT=xt[:, c, i * 128:(i + 1) * 128],
                   rhs=wv_sb[:, c, :], start=(c == 0), stop=(c == DC - 1))
            nc.vector.tensor_copy(
                out=v_sb[:, st, :, 0:64],
                in_=vp.rearrange("p (h e) -> p h e", h=HG))

    with tc.tile_pool(name="ps_pj", bufs=2, space="PSUM") as pj_pool:
        xt = load_xt(xt_pool, io["xk"], 0, "xtk0")
        # remaining weights while the first k chunk is in flight
        nc.sync.dma_start(out=wv_sb, in_=io["wv"][:])
        nc.sync.dma_start(out=wq_sb, in_=io["wq"][:])
        nc.sync.dma_start(out=wo_sb, in_=io["wo"][:])
        nc.sync.dma_start(out=bq_sb, in_=io["bq"][:])
        proj_qk(xt, wk_sb, bk_sb, kT, 0, pj_pool, "pj")
        for lc in range(1, 4):
            xt = load_xt(xt_pool, io["xk"], lc, f"xtk{lc}")
            proj_qk(xt, wk_sb, bk_sb, kT, lc, pj_pool, "pj")
        for lc in range(4):
            xt = load_xt(xt_pool, io["xv"], lc, f"xtv{lc}")
            proj_v(xt, lc, pj_pool)
        xt = load_xt(xq_pool, io["xq"], 0, "xtq0")
        proj_qk(xt, wq_sb, bq_sb, qT, 0, pj_pool, "pj")
        xq_next = load_xt(xq_pool, io["xq"], 1, "xtq1")

    # ---- attention ------------------------------------------------------
    # PSUM budget (8 banks): scA 4 + scB 2 + av/op/pj 2.
    ps_scA = ctx.enter_context(tc.tile_pool(name="ps_scA", bufs=1, space="PSUM"))
    ps_scB = ctx.enter_context(tc.tile_pool(name="ps_scB", bufs=1, space="PSUM"))
    ps_av = ctx.enter_context(tc.tile_pool(name="ps_av", bufs=2, space="PSUM"))

    pending = []     # deferred emissions, flushed inside the next block

    def flush_pending():
        for f in pending:
            f()
        pending.clear()

    def defer_normalize(av, attnT, lq, ec):
        def norm():
            for hh in range(2):
                rz = rz_pool.tile([64, 512], F16, tag="rz",
                                  name=f"rz{lq}_{ec}_{hh}")
                with nc.allow_low_precision(reason="1/Z in f16"):
                    nc.vector.reciprocal(rz, av[hh][64:128, :])
                nc.vector.tensor_mul(
                    out=attnT[hh * 64:hh * 64 + 64, ec, :],
                    in0=av[hh][0:64, :], in1=rz)
        pending.append(norm)

    def defer_outproj(attnT, lq):
        def op_emit():
            for i in range(4):
                lt = lq * 4 + i
                op = ps_av.tile([128, D], F32, tag="av", name=f"op_{lq}_{i}")
                for g2 in range(2):
                    mm(op, lhsT=attnT[:, g2, i * 128:(i + 1) * 128],
                       rhs=wo_sb[:, g2, :], start=(g2 == 0), stop=(g2 == 1))
                ob = ob_pool.tile([128, D], F16, tag="ob", name=f"ob_{lq}_{i}")
                nc.vector.tensor_copy(out=ob, in_=op)
                nc.sync.dma_start(out=io["out"][lt * 128:(lt + 1) * 128, :],
                                  in_=ob)
        pending.append(op_emit)

    def defer_qproj(lq):
        # project qT for l-quarter lq+1 (xq chunk already loading) and
        # kick off the chunk DMA for lq+2.
        def qp():
            nonlocal xq_next
            proj_qk(xq_next, wq_sb, bq_sb, qT, lq + 1, ps_av, "av")
            if lq + 2 < LQ:
                xq_next = load_xt(xq_pool, io["xq"], lq + 2, f"xtq{lq + 2}")
        pending.append(qp)

    for lq in range(LQ):
        l0 = lq * 512
        attnT = at_pool.tile([128, 2, 512], F16, tag="at", name=f"at{lq}")
        for ec in range(2):
            av = None
            for gi, (u0, n) in enumerate(GROUPS):
                a_side = (gi % 2 == 0)
                pool = ps_scA if a_side else ps_scB
                full = pool.tile([128, 4 if a_side else 2, 512], F32,
                                 tag="sc", name=f"sc{lq}_{ec}_{gi}")
                sc = full[:, 0:n, :]
                for k in range(n):
                    j, hh = (u0 + k) // 2, (u0 + k) % 2
                    o = hh * 64
                    mm(sc[:, k, :],
                       lhsT=kT[o:o + 64, ec, j * 128:(j + 1) * 128],
                       rhs=qT[o:o + 64, ec, l0:l0 + 512],
                       start=True, stop=True, tile_position=(o, 0))
                ep_full = e_pool.tile([128, 4, 512], F16, tag="ep",
                                      name=f"ep{lq}_{ec}_{gi}")
                ep = ep_full[:, 0:n, :]
                nc.scalar.activation(out=ep, in_=sc,
                                     func=mybir.ActivationFunctionType.Exp,
                                     bias=shift_col[:, 0:1], scale=1.0)
                if gi == 0:
                    # deferred work from the previous block fills the
                    # PE's exp-wait gap; alloc av AFTER it so the PSUM
                    # rotation sees the deferred readers.
                    flush_pending()
                    av = [ps_av.tile([128, 512], F32, tag="av",
                                     name=f"av{lq}_{ec}_{i}")
                          for i in range(2)]
                for k in range(n):
                    j, hh = (u0 + k) // 2, (u0 + k) % 2
                    mm(av[hh], lhsT=v_sb[:, j, 2 * ec + hh, :],
                       rhs=ep[:, k, :], start=(j == 0), stop=(j == ST - 1))
            defer_normalize(av, attnT, lq, ec)
            if ec == 0:
                # q-proj for lq+1 flushes at (lq, ec=1) g0 -- must land
                # before block (lq+1, ec=0)'s scores read that qT range.
                if lq + 1 < LQ:
                    defer_qproj(lq)
            else:
                defer_outproj(attnT, lq)
    flush_pending()


def build_nc():
    nc = bacc.Bacc()
    io = {}
    io["xq"] = nc.declare_dram_parameter("xq", [128, DC, L], F16, isOutput=False)
    io["xk"] = nc.declare_dram_parameter("xk", [128, DC, S], F16, isOutput=False)
    io["xv"] = nc.declare_dram_parameter("xv", [128, DC, S], F16, isOutput=False)
    io["wq"] = nc.declare_dram_parameter("wq", [128, DC, EG], F16, isOutput=False)
    io["wk"] = nc.declare_dram_parameter("wk", [128, DC, EG], F16, isOutput=False)
    io["wv"] = nc.declare_dram_parameter("wv", [128, DC, EG], F16, isOutput=False)
    io["wo"] = nc.declare_dram_parameter("wo", [128, 2, D], F16, isOutput=False)
    io["bq"] = nc.declare_dram_parameter("bq", [128, 2], F32, isOutput=False)
    io["bk"] = nc.declare_dram_parameter("bk", [128, 2], F32, isOutput=False)
    io["out"] = nc.declare_dram_parameter("out", [L, D], F16, isOutput=True)
    with tile.TileContext(nc) as tc:
        with ExitStack() as ctx:
            _emit(ctx, tc, io)
    nc.compile()
    return nc


_NC = None


def _get_nc():
    global _NC
    if _NC is None:
        _NC = build_nc()
    return _NC


def _chunk_w(w):
    """[512, n] -> [128, 4, n] fp16:  [p, c, :] = w[128c+p, :]"""
    n = w.shape[1]
    return np.ascontiguousarray(
        w.reshape(DC, 128, n).transpose(1, 0, 2), dtype=np.float16)


def _chunk_xt(x):
    """[2048, 512] -> X^T chunked [128, 4, 2048] fp16: [p, c, l] = x[l, 128c+p]"""
    return np.ascontiguousarray(
        np.asarray(x, dtype=np.float16).T.reshape(DC, 128, L).transpose(1, 0, 2))


def make_in_maps(queries, keys, values, tau, Wq, bq, Wk, bk, Wv, bv, Wo):
    xts = [{"xq": _chunk_xt(queries[b]), "xk": _chunk_xt(keys[b]),
            "xv": _chunk_xt(values[b])} for b in range(B)]
    in_maps = []
    for c in range(N_CORES):
        b, g = c // 2, c % 2
        e0 = g * EG
        f = np.float32(SCALE * tau[b])
        wq = _chunk_w(Wq[:, e0:e0 + EG] * f)
        wk = _chunk_w(Wk[:, e0:e0 + EG])
        wv = _chunk_w(Wv[:, e0:e0 + EG])
        wo = np.ascontiguousarray(
            Wo[e0:e0 + EG, :].reshape(2, 128, D).transpose(1, 0, 2),
            dtype=np.float16)
        in_maps.append({
            **xts[b],
            "wq": wq, "wk": wk, "wv": wv, "wo": wo,
            "bq": np.ascontiguousarray(
                (bq[e0:e0 + EG] * f).reshape(2, 128).T, dtype=np.float32),
            "bk": np.ascontiguousarray(
                bk[e0:e0 + EG].reshape(2, 128).T, dtype=np.float32),
        })
    return in_maps


def kernel(queries, keys, values, tau, delta, Wq, bq, Wk, bk, Wv, bv, Wo, bo,
           **_unused):
    queries = np.asarray(queries, dtype=np.float32)
    keys = np.asarray(keys, dtype=np.float32)
    values = np.asarray(values, dtype=np.float32)
    tau = np.asarray(tau, dtype=np.float32)
    Wq, bq = np.asarray(Wq, np.float32), np.asarray(bq, np.float32)
    Wk, bk = np.asarray(Wk, np.float32), np.asarray(bk, np.float32)
    Wv, bv = np.asarray(Wv, np.float32), np.asarray(bv, np.float32)
    Wo, bo = np.asarray(Wo, np.float32), np.asarray(bo, np.float32)

    nc = _get_nc()
    in_maps = make_in_maps(queries, keys, values, tau, Wq, bq, Wk, bk, Wv, bv, Wo)
    res = run_bass_kernel_spmd(nc, in_maps, list(range(N_CORES)))
    # attn rows sum to 1 -> +bv flows through Wo as a constant row; + bo.
    const_row = (bv @ Wo + bo).astype(np.float32)  # [512]
    out = np.empty((B, L, D), dtype=np.float32)
    for b in range(B):
        out[b] = res.results[2 * b]["out"].astype(np.float32) \
            + res.results[2 * b + 1]["out"].astype(np.float32) + const_row
    return out


if __name__ == "__main__":
    nc = build_nc()
    print("built OK")


# revision 8
# speedup vs baseline: 1.1980x; 1.1396x over previous
"""DSAttention layer for Trainium2, 8 NeuronCores.

Sharding: core c -> batch b = c//2, head-group g = c%2 (4 heads each,
e-columns 256g..256g+255 of the 512-wide head dim).  tau[b]/8 is folded
into each core's Wq/bq slice on the host; delta[b] broadcasts over the
softmax axis and drops out exactly.  Each core emits its head-group's
partial output projection [2048, 512] fp16; the host sums the pair per
batch in fp32 and adds (bv @ Wo + bo).

Host pre-transposes X^T (d-major, fp16): zero PE transposes on device.

Device dataflow per core (fp16 matmul operands, fp32 PSUM accum):
  qT/kT [e 256, l 2048] = W^T @ X^T   (e on partitions)
  v_aug [s, 128] per (s-tile, head): cols 0-63 = V, cols 64-127 ones
        -> AV rows 64-127 all equal the softmax denominator Z.
  scoresT = kT.T @ qT per head, head pairs on row groups 0-63/64-127,
        into PSUM exp-groups of 4 (scA) / 2 (scB) [128,512] units
  E = exp(scoresT - 2) fp16, ONE ACT instr per group (amortize the
        ~352-cycle ACT overhead); A/B alternation double-buffers in
        the 8-bank PSUM budget (4 scA + 2 scB + 2 av).
  av = v_aug.T @ E accumulated over 16 s-tiles -> copy av to SBUF f16
        EARLY (frees the PSUM bank in one DVE op), then 1/Z (f16 DVE
        reciprocal) and one f16 mul -> attnT, all off the hot path.
  out[l,512] = attnT.T @ Wo  (K=128: head pairs packed on partitions)

Scheduling: all deferred work (av copy, normalize, output projection,
next q-projection) is queued as SLOTS and drained ONE PER GROUP inside
the next block, so PE-side work lands in the PE's exp-wait gaps and
never starves the scalar engine.  Aux psums ride the scB rotation;
the prolog round-robins its psums over the attention pools.
"""

import numpy as np
from contextlib import ExitStack

import concourse.bass as bass
import concourse.bacc as bacc
import concourse.mybir as mybir
import concourse.tile as tile
from concourse.bass_utils import run_bass_kernel_spmd

F32 = mybir.dt.float32
F16 = mybir.dt.float16

B, L, S, D = 4, 2048, 2048, 512
H, E = 8, 64
HG = 4                # heads per core
EG = HG * E           # 256
N_CORES = 8

ST = S // 128         # 16 s-tiles
DC = D // 128         # 4 d-chunks
LQ = 4                # l-quarters of 512
SCALE = 1.0 / np.sqrt(np.float32(E))
EXP_SHIFT = -2.0

# unit u = (j, hh): j = u//2 (s-tile), hh = u%2 (head in pair).
# groups alternate scA(4 units)/scB(2 units); last group rides scA.
GROUPS = []
_u = 0
while _u < 32:
    _n = min(4 if (len(GROUPS) % 2 == 0) else 2, 32 - _u)
    GROUPS.append((_u, _n))
    _u += _n


def _emit(ctx: ExitStack, tc: "tile.TileContext", io: dict):
    nc = tc.nc
    mm = nc.tensor.matmul

    singles = ctx.enter_context(tc.tile_pool(name="singles", bufs=1))
    bigs = ctx.enter_context(tc.tile_pool(name="bigs", bufs=1))
    xt_pool = ctx.enter_context(tc.tile_pool(name="xt", bufs=2))
    xq_pool = ctx.enter_context(tc.tile_pool(name="xq", bufs=2))
    e_pool = ctx.enter_context(tc.tile_pool(name="eslab", bufs=3))
    avc_pool = ctx.enter_context(tc.tile_pool(name="avc", bufs=4))
    rz_pool = ctx.enter_context(tc.tile_pool(name="rz", bufs=4))
    at_pool = ctx.enter_context(tc.tile_pool(name="attnT", bufs=2))
    ob_pool = ctx.enter_context(tc.tile_pool(name="outsb", bufs=3))

    ps_scA = ctx.enter_context(tc.tile_pool(name="ps_scA", bufs=1, space="PSUM"))
    ps_scB = ctx.enter_context(tc.tile_pool(name="ps_scB", bufs=1, space="PSUM"))
    ps_av = ctx.enter_context(tc.tile_pool(name="ps_av", bufs=2, space="PSUM"))

    # ---- constants & weights -------------------------------------------
    shift_col = singles.tile([128, 1], F32)
    nc.vector.memset(shift_col, EXP_SHIFT)
    warm = singles.tile([128, 1], F16)

    wq_sb = singles.tile([128, DC, EG], F16)
    wk_sb = singles.tile([128, DC, EG], F16)
    wv_sb = singles.tile([128, DC, EG], F16)
    wo_sb = singles.tile([128, 2, D], F16)     # [p, g2, n] = Wo[128*g2+p, n]
    bq_sb = singles.tile([128, 2], F32)
    bk_sb = singles.tile([128, 2], F32)
    nc.sync.dma_start(out=wk_sb, in_=io["wk"][:])
    nc.sync.dma_start(out=bk_sb, in_=io["bk"][:])

    # warm the ACT exp table-set (~2.7us) during the prolog
    nc.scalar.activation(out=warm, in_=shift_col,
                         func=mybir.ActivationFunctionType.Exp,
                         bias=shift_col[:, 0:1], scale=1.0)

    # ---- big persistent SBUF tensors -----------------------------------
    qT = bigs.tile([128, 2, L], F16, tag="qT")
    kT = bigs.tile([128, 2, S], F16, tag="kT")
    v_sb = bigs.tile([128, ST, HG, 128], F16, tag="v")
    nc.vector.memset(v_sb[:, :, :, 64:128], 1.0)  # ones cols -> Z rows

    # round-robin psum provider (prolog + aux slots ride these pools)
    _rr = [0]

    def pj_psum(nm):
        r = _rr[0] % 3
        _rr[0] += 1
        if r == 0:
            t = ps_scA.tile([128, 4, 512], F32, tag="sc", name=nm)
            return t[:, 0, :]
        if r == 1:
            t = ps_scB.tile([128, 2, 512], F32, tag="sc", name=nm)
            return t[:, 0, :]
        return ps_av.tile([128, 512], F32, tag="av", name=nm)

    def aux_psum(nm):
        # aux work inside attention rides the scB rotation only
        t = ps_scB.tile([128, 2, 512], F32, tag="sc", name=nm)
        return t[:, 0, :]

    # ---- projections ----------------------------------------------------
    def load_xt(pool, x_dram, lc, nm):
        xt = pool.tile([128, DC, 512], F16, tag="xt", name=nm)
        nc.sync.dma_start(out=xt, in_=x_dram[:, :, lc * 512:(lc + 1) * 512])
        return xt

    def proj_qk_ec(xt, w_sb, b_sb, dst, lc, ec, psum):
        for c in range(DC):
            mm(psum, lhsT=w_sb[:, c, ec * 128:(ec + 1) * 128],
               rhs=xt[:, c, :], start=(c == 0), stop=(c == DC - 1))
        nc.vector.tensor_scalar_add(
            out=dst[:, ec, lc * 512:(lc + 1) * 512], in0=psum,
            scalar1=b_sb[:, ec:ec + 1])

    def proj_v(xt, lc):
        for i in range(4):
            st = lc * 4 + i
            vp = pj_psum(f"vp_{lc}_{i}")[:, 0:EG]
            for c in range(DC):
                mm(vp, lhsT=xt[:, c, i * 128:(i + 1) * 128],
                   rhs=wv_sb[:, c, :], start=(c == 0), stop=(c == DC - 1))
            nc.vector.tensor_copy(
                out=v_sb[:, st, :, 0:64],
                in_=vp.rearrange("p (h e) -> p h e", h=HG))

    # ---- prolog: k fully, v fully, q quarter 0 --------------------------
    xt = load_xt(xt_pool, io["xk"], 0, "xtk0")
    nc.sync.dma_start(out=wv_sb, in_=io["wv"][:])
    nc.sync.dma_start(out=wq_sb, in_=io["wq"][:])
    nc.sync.dma_start(out=wo_sb, in_=io["wo"][:])
    nc.sync.dma_start(out=bq_sb, in_=io["bq"][:])
    for ec in range(2):
        proj_qk_ec(xt, wk_sb, bk_sb, kT, 0, ec, pj_psum(f"ppk0_{ec}"))
    for lc in range(1, 4):
        xt = load_xt(xt_pool, io["xk"], lc, f"xtk{lc}")
        for ec in range(2):
            proj_qk_ec(xt, wk_sb, bk_sb, kT, lc, ec, pj_psum(f"ppk{lc}_{ec}"))
    for lc in range(4):
        xt = load_xt(xt_pool, io["xv"], lc, f"xtv{lc}")
        proj_v(xt, lc)
    xt = load_xt(xq_pool, io["xq"], 0, "xtq0")
    for ec in range(2):
        proj_qk_ec(xt, wq_sb, bq_sb, qT, 0, ec, pj_psum(f"ppq0_{ec}"))
    state = {"xq_next": load_xt(xq_pool, io["xq"], 1, "xtq1")}

    # ---- attention with slot-spread deferred work -----------------------
    pending = []     # closures from the previous block, drained 1/group

    def slot_avcopy(av, lq, ec):
        def f():
            avc = []
            for hh in range(2):
                t = avc_pool.tile([128, 512], F16, tag="avc",
                                  name=f"avc{lq}_{ec}_{hh}")
                nc.vector.tensor_copy(out=t, in_=av[hh][:])
                avc.append(t)
            slot_avcopy.out = avc
        return f

    def slot_norm(attnT, lq, ec, hh):
        def f():
            avc = slot_avcopy.out
            rz = rz_pool.tile([64, 512], F16, tag="rz",
                              name=f"rz{lq}_{ec}_{hh}")
            with nc.allow_low_precision(reason="1/Z in f16"):
                nc.vector.reciprocal(rz, avc[hh][64:128, :])
            nc.vector.tensor_mul(
                out=attnT[hh * 64:hh * 64 + 64, ec, :],
                in0=avc[hh][0:64, :], in1=rz)
        return f

    def slot_outproj(attnT, lq, i):
        def f():
            lt = lq * 4 + i
            op = aux_psum(f"op_{lq}_{i}")
            for g2 in range(2):
                mm(op, lhsT=attnT[:, g2, i * 128:(i + 1) * 128],
                   rhs=wo_sb[:, g2, :], start=(g2 == 0), stop=(g2 == 1))
            ob = ob_pool.tile([128, D], F16, tag="ob", name=f"ob_{lq}_{i}")
            nc.vector.tensor_copy(out=ob, in_=op)
            nc.sync.dma_start(out=io["out"][lt * 128:(lt + 1) * 128, :], in_=ob)
        return f

    def slot_qproj(lq, ec):
        def f():
            proj_qk_ec(state["xq_next"], wq_sb, bq_sb, qT, lq + 1, ec,
                       aux_psum(f"ppq{lq + 1}_{ec}"))
            if ec == 1 and lq + 2 < LQ:
                state["xq_next"] = load_xt(xq_pool, io["xq"], lq + 2,
                                           f"xtq{lq + 2}")
        return f

    for lq in range(LQ):
        l0 = lq * 512
        attnT = at_pool.tile([128, 2, 512], F16, tag="at", name=f"at{lq}")
        for ec in range(2):
            slots, pending = pending, []
            av = None
            for gi, (u0, n) in enumerate(GROUPS):
                a_side = (gi % 2 == 0)
                pool = ps_scA if a_side else ps_scB
                full = pool.tile([128, 4 if a_side else 2, 512], F32,
                                 tag="sc", name=f"sc{lq}_{ec}_{gi}")
                sc = full[:, 0:n, :]
                for k in range(n):
                    j, hh = (u0 + k) // 2, (u0 + k) % 2
                    o = hh * 64
                    mm(sc[:, k, :],
                       lhsT=kT[o:o + 64, ec, j * 128:(j + 1) * 128],
                       rhs=qT[o:o + 64, ec, l0:l0 + 512],
                       start=True, stop=True, tile_position=(o, 0))
                ep_full = e_pool.tile([128, 4, 512], F16, tag="ep",
                                      name=f"ep{lq}_{ec}_{gi}")
                ep = ep_full[:, 0:n, :]
                nc.scalar.activation(out=ep, in_=sc,
                                     func=mybir.ActivationFunctionType.Exp,
                                     bias=shift_col[:, 0:1], scale=1.0)
                if gi < len(slots):
                    slots[gi]()
                if gi == 0:
                    av = [ps_av.tile([128, 512], F32, tag="av",
                                     name=f"av{lq}_{ec}_{i}")
                          for i in range(2)]
                for k in range(n):
                    j, hh = (u0 + k) // 2, (u0 + k) % 2
                    mm(av[hh], lhsT=v_sb[:, j, 2 * ec + hh, :],
                       rhs=ep[:, k, :], start=(j == 0), stop=(j == ST - 1))
            pending.append(slot_avcopy(av, lq, ec))
            pending.append(slot_norm(attnT, lq, ec, 0))
            pending.append(slot_norm(attnT, lq, ec, 1))
            if ec == 0:
                if lq + 1 < LQ:
                    pending.append(slot_qproj(lq, 0))
                    pending.append(slot_qproj(lq, 1))
            else:
                for i in range(4):
                    pending.append(slot_outproj(attnT, lq, i))
    for f in pending:
        f()


def build_nc():
    nc = bacc.Bacc()
    io = {}
    io["xq"] = nc.declare_dram_parameter("xq", [128, DC, L], F16, isOutput=False)
    io["xk"] = nc.declare_dram_parameter("xk", [128, DC, S], F16, isOutput=False)
    io["xv"] = nc.declare_dram_parameter("xv", [128, DC, S], F16, isOutput=False)
    io["wq"] = nc.declare_dram_parameter("wq", [128, DC, EG], F16, isOutput=False)
    io["wk"] = nc.declare_dram_parameter("wk", [128, DC, EG], F16, isOutput=False)
    io["wv"] = nc.declare_dram_parameter("wv", [128, DC, EG], F16, isOutput=False)
    io["wo"] = nc.declare_dram_parameter("wo", [128, 2, D], F16, isOutput=False)
    io["bq"] = nc.declare_dram_parameter("bq", [128, 2], F32, isOutput=False)
    io["bk"] = nc.declare_dram_parameter("bk", [128, 2], F32, isOutput=False)
    io["out"] = nc.declare_dram_parameter("out", [L, D], F16, isOutput=True)
    with tile.TileContext(nc) as tc:
        with ExitStack() as ctx:
            _emit(ctx, tc, io)
    nc.compile()
    return nc


_NC = None


def _get_nc():
    global _NC
    if _NC is None:
        _NC = build_nc()
    return _NC


def _chunk_w(w):
    n = w.shape[1]
    return np.ascontiguousarray(
        w.reshape(DC, 128, n).transpose(1, 0, 2), dtype=np.float16)


def _chunk_xt(x):
    return np.ascontiguousarray(
        np.asarray(x, dtype=np.float16).T.reshape(DC, 128, L).transpose(1, 0, 2))


def make_in_maps(queries, keys, values, tau, Wq, bq, Wk, bk, Wv, bv, Wo):
    xts = [{"xq": _chunk_xt(queries[b]), "xk": _chunk_xt(keys[b]),
            "xv": _chunk_xt(values[b])} for b in range(B)]
    in_maps = []
    for c in range(N_CORES):
        b, g = c // 2, c % 2
        e0 = g * EG
        f = np.float32(SCALE * tau[b])
        in_maps.append({
            **xts[b],
            "wq": _chunk_w(Wq[:, e0:e0 + EG] * f),
            "wk": _chunk_w(Wk[:, e0:e0 + EG]),
            "wv": _chunk_w(Wv[:, e0:e0 + EG]),
            "wo": np.ascontiguousarray(
                Wo[e0:e0 + EG, :].reshape(2, 128, D).transpose(1, 0, 2),
                dtype=np.float16),
            "bq": np.ascontiguousarray(
                (bq[e0:e0 + EG] * f).reshape(2, 128).T, dtype=np.float32),
            "bk": np.ascontiguousarray(
                bk[e0:e0 + EG].reshape(2, 128).T, dtype=np.float32),
        })
    return in_maps


def kernel(queries, keys, values, tau, delta, Wq, bq, Wk, bk, Wv, bv, Wo, bo,
           **_unused):
    queries = np.asarray(queries, dtype=np.float32)
    keys = np.asarray(keys, dtype=np.float32)
    values = np.asarray(values, dtype=np.float32)
    tau = np.asarray(tau, dtype=np.float32)
    Wq, bq = np.asarray(Wq, np.float32), np.asarray(bq, np.float32)
    Wk, bk = np.asarray(Wk, np.float32), np.asarray(bk, np.float32)
    Wv, bv = np.asarray(Wv, np.float32), np.asarray(bv, np.float32)
    Wo, bo = np.asarray(Wo, np.float32), np.asarray(bo, np.float32)

    nc = _get_nc()
    in_maps = make_in_maps(queries, keys, values, tau, Wq, bq, Wk, bk, Wv, bv, Wo)
    res = run_bass_kernel_spmd(nc, in_maps, list(range(N_CORES)))
    const_row = (bv @ Wo + bo).astype(np.float32)  # [512]
    out = np.empty((B, L, D), dtype=np.float32)
    for b in range(B):
        out[b] = res.results[2 * b]["out"].astype(np.float32) \
            + res.results[2 * b + 1]["out"].astype(np.float32) + const_row
    return out


if __name__ == "__main__":
    nc = build_nc()
    print("built OK")


# revision 13
# speedup vs baseline: 1.3405x; 1.1189x over previous
"""DSAttention layer for Trainium2, 8 NeuronCores.

Sharding: core c -> batch b = c//2, head-group g = c%2 (4 heads each,
e-columns 256g..256g+255 of the 512-wide head dim).  tau[b]/8 is folded
into each core's Wq/bq slice on the host; delta[b] broadcasts over the
softmax axis and drops out exactly.  Each core emits its head-group's
partial output projection [2048, 512] fp16; the host sums the pair per
batch in fp32 and adds (bv @ Wo + bo).

Host pre-transposes X^T (d-major, fp16): zero PE transposes on device.

Device dataflow per core (fp16 matmul operands, fp32 PSUM accum):
  qT/kT [e 256, l 2048] = W^T @ X^T   (e on partitions)
  v_aug [s, 128] per (s-tile, head): cols 0-63 = V, cols 64-127 ones
        -> AV rows 64-127 all equal the softmax denominator Z.
  scoresT = kT.T @ qT per head, head pairs on row groups 0-63/64-127,
        into PSUM exp-groups of 4 (scA) / 2 (scB) [128,512] units
  E = exp(scoresT - 2) fp16, ONE ACT instr per group (amortize the
        ~352-cycle ACT overhead); A/B alternation double-buffers in
        the 8-bank PSUM budget (4 scA + 2 scB + 2 av).
  av = v_aug.T @ E accumulated over 16 s-tiles -> copy av to SBUF f16
        EARLY (frees the PSUM bank in one DVE op), then 1/Z (f16 DVE
        reciprocal) and one f16 mul -> attnT, all off the hot path.
  out[l,512] = attnT.T @ Wo  (K=128: head pairs packed on partitions)

Scheduling: all deferred work (av copy, normalize, output projection,
next q-projection) is queued as SLOTS and drained ONE PER GROUP inside
the next block, so PE-side work lands in the PE's exp-wait gaps and
never starves the scalar engine.  Aux psums ride the scB rotation;
the prolog round-robins its psums over the attention pools.
"""

import numpy as np
from contextlib import ExitStack

import concourse.bass as bass
import concourse.bacc as bacc
import concourse.mybir as mybir
import concourse.tile as tile
from concourse.bass_utils import run_bass_kernel_spmd

F32 = mybir.dt.float32
F16 = mybir.dt.float16

B, L, S, D = 4, 2048, 2048, 512
H, E = 8, 64
HG = 4                # heads per core
EG = HG * E           # 256
N_CORES = 8

ST = S // 128         # 16 s-tiles
DC = D // 128         # 4 d-chunks
LQ = 4                # l-quarters of 512
SCALE = 1.0 / np.sqrt(np.float32(E))
EXP_SHIFT = -2.0

# unit u = (j, hh): j = u//2 (s-tile), hh = u%2 (head in pair).
# groups alternate scA(4 units)/scB(2 units).  Even blocks start on A
# and end on A; odd blocks start and end on B — so every block
# boundary crosses pools and the next block's first scores never wait
# for the previous block's last exp (scX bufs=1 double-buffering).
def _mk_groups(start_a):
    gs, u = [], 0
    while u < 32:
        a = (len(gs) % 2 == 0) == start_a
        n = min(4 if a else 2, 32 - u)
        gs.append((u, n, a))
        u += n
    return gs


GROUPS_EVEN = _mk_groups(True)    # A4 B2 ... A2
GROUPS_ODD = _mk_groups(False)    # B2 A4 ... B2


def _emit(ctx: ExitStack, tc: "tile.TileContext", io: dict):
    nc = tc.nc
    mm = nc.tensor.matmul

    singles = ctx.enter_context(tc.tile_pool(name="singles", bufs=1))
    bigs = ctx.enter_context(tc.tile_pool(name="bigs", bufs=1))
    xt_pool = ctx.enter_context(tc.tile_pool(name="xt", bufs=2))
    xq_pool = ctx.enter_context(tc.tile_pool(name="xq", bufs=2))
    e_pool = ctx.enter_context(tc.tile_pool(name="eslab", bufs=3))
    avc_pool = ctx.enter_context(tc.tile_pool(name="avc", bufs=4))
    rz_pool = ctx.enter_context(tc.tile_pool(name="rz", bufs=4))
    at_pool = ctx.enter_context(tc.tile_pool(name="attnT", bufs=2))
    ob_pool = ctx.enter_context(tc.tile_pool(name="outsb", bufs=3))

    ps_scA = ctx.enter_context(tc.tile_pool(name="ps_scA", bufs=1, space="PSUM"))
    ps_scB = ctx.enter_context(tc.tile_pool(name="ps_scB", bufs=1, space="PSUM"))
    ps_av = ctx.enter_context(tc.tile_pool(name="ps_av", bufs=2, space="PSUM"))

    # ---- constants & weights -------------------------------------------
    shift_col = singles.tile([128, 1], F32)
    nc.vector.memset(shift_col, EXP_SHIFT)
    warm = singles.tile([128, 1], F16)

    wq_sb = singles.tile([128, DC, EG], F16)
    wk_sb = singles.tile([128, DC, EG], F16)
    wv_sb = singles.tile([128, DC, EG], F16)
    wo_sb = singles.tile([128, 2, D], F16)     # [p, g2, n] = Wo[128*g2+p, n]
    bq_sb = singles.tile([128, 2], F32)
    bk_sb = singles.tile([128, 2], F32)
    nc.sync.dma_start(out=wk_sb, in_=io["wk"][:])
    nc.sync.dma_start(out=bk_sb, in_=io["bk"][:])

    # warm the ACT exp table-set (~2.7us) during the prolog
    nc.scalar.activation(out=warm, in_=shift_col,
                         func=mybir.ActivationFunctionType.Exp,
                         bias=shift_col[:, 0:1], scale=1.0)

    # ---- big persistent SBUF tensors -----------------------------------
    qT = bigs.tile([128, 2, L], F16, tag="qT")
    kT = bigs.tile([128, 2, S], F16, tag="kT")
    v_sb = bigs.tile([128, ST, HG, 128], F16, tag="v")
    nc.vector.memset(v_sb[:, :, :, 64:128], 1.0)  # ones cols -> Z rows

    # round-robin psum provider (prolog + aux slots ride these pools)
    _rr = [0]

    def pj_psum(nm):
        r = _rr[0] % 3
        _rr[0] += 1
        if r == 0:
            t = ps_scA.tile([128, 4, 512], F32, tag="sc", name=nm)
            return t[:, 0, :]
        if r == 1:
            t = ps_scB.tile([128, 2, 512], F32, tag="sc", name=nm)
            return t[:, 0, :]
        return ps_av.tile([128, 512], F32, tag="av", name=nm)

    def aux_psum(nm):
        # aux work inside attention rides the scB rotation only
        t = ps_scB.tile([128, 2, 512], F32, tag="sc", name=nm)
        return t[:, 0, :]

    # ---- projections ----------------------------------------------------
    def load_xt(pool, x_dram, lc, nm):
        xt = pool.tile([128, DC, 512], F16, tag="xt", name=nm)
        nc.sync.dma_start(out=xt, in_=x_dram[:, :, lc * 512:(lc + 1) * 512])
        return xt

    def proj_qk_ec(xt, w_sb, b_sb, dst, lc, ec, psum):
        for c in range(DC):
            mm(psum, lhsT=w_sb[:, c, ec * 128:(ec + 1) * 128],
               rhs=xt[:, c, :], start=(c == 0), stop=(c == DC - 1))
        nc.vector.tensor_scalar_add(
            out=dst[:, ec, lc * 512:(lc + 1) * 512], in0=psum,
            scalar1=b_sb[:, ec:ec + 1])

    def proj_v_st(xt, st, psum):
        i = st % 4
        vp = psum[:, 0:EG]
        for c in range(DC):
            mm(vp, lhsT=xt[:, c, i * 128:(i + 1) * 128],
               rhs=wv_sb[:, c, :], start=(c == 0), stop=(c == DC - 1))
        nc.vector.tensor_copy(
            out=v_sb[:, st, :, 0:64],
            in_=vp.rearrange("p (h e) -> p h e", h=HG))

    # ---- prolog: k lc0-1, v st0-4, q lc0; the rest fuses into block 0 ---
    xv_pool = ctx.enter_context(tc.tile_pool(name="xv", bufs=2))
    xtk0 = load_xt(xt_pool, io["xk"], 0, "xtk0")
    nc.sync.dma_start(out=wv_sb, in_=io["wv"][:])
    nc.sync.dma_start(out=wq_sb, in_=io["wq"][:])
    nc.sync.dma_start(out=wo_sb, in_=io["wo"][:])
    nc.sync.dma_start(out=bq_sb, in_=io["bq"][:])
    for ec in range(2):
        proj_qk_ec(xtk0, wk_sb, bk_sb, kT, 0, ec, pj_psum(f"ppk0_{ec}"))
    xtk1 = load_xt(xt_pool, io["xk"], 1, "xtk1")
    for ec in range(2):
        proj_qk_ec(xtk1, wk_sb, bk_sb, kT, 1, ec, pj_psum(f"ppk1_{ec}"))
    xv0 = load_xt(xv_pool, io["xv"], 0, "xtv0")
    for st in range(4):
        proj_v_st(xv0, st, pj_psum(f"vp_{st}"))
    xv1 = load_xt(xv_pool, io["xv"], 1, "xtv1")
    proj_v_st(xv1, 4, pj_psum("vp_4"))
    xtq = load_xt(xq_pool, io["xq"], 0, "xtq0")
    for ec in range(2):
        proj_qk_ec(xtq, wq_sb, bq_sb, qT, 0, ec, pj_psum(f"ppq0_{ec}"))
    state = {"xq_next": load_xt(xq_pool, io["xq"], 1, "xtq1"),
             "xk_next": load_xt(xt_pool, io["xk"], 2, "xtk2"),
             "xv_cur": xv1, "xv_cur_lc": 1, "xv_next": None}

    # ---- attention with slot-spread deferred work -----------------------
    # block 0 absorbs the remaining k/v projections via its slots
    def slot_kproj(lc):
        def f():
            xt = state["xk_next"]
            if lc == 2:
                state["xk_next"] = load_xt(xt_pool, io["xk"], 3, "xtk3")
            for ec in range(2):
                proj_qk_ec(xt, wk_sb, bk_sb, kT, lc, ec,
                           aux_psum(f"ppk{lc}_{ec}"))
        return f

    def slot_vproj(sts, load_lc):
        def f():
            for st in sts:
                lc = st // 4
                if lc > state.get("xv_cur_lc", 1):
                    state["xv_cur"] = state["xv_next"]
                    state["xv_cur_lc"] = lc
                proj_v_st(state["xv_cur"], st, aux_psum(f"vp_{st}"))
            if load_lc is not None:
                state["xv_next"] = load_xt(xv_pool, io["xv"], load_lc,
                                           f"xtv{load_lc}")
        return f

    pending = [
        slot_kproj(2),
        slot_kproj(3),
        slot_vproj([5], None),
        slot_vproj([6, 7], 2),
        slot_vproj([8, 9], None),
        slot_vproj([10, 11], 3),
        slot_vproj([12, 13], None),
        slot_vproj([14, 15], None),
    ]

    def slot_avcopy(av, lq, ec):
        def f():
            avc = []
            for hh in range(2):
                t = avc_pool.tile([128, 512], F16, tag="avc",
                                  name=f"avc{lq}_{ec}_{hh}")
                nc.vector.tensor_copy(out=t, in_=av[hh][:])
                avc.append(t)
            slot_avcopy.out = avc
        return f

    def slot_norm(attnT, lq, ec, hh):
        def f():
            avc = slot_avcopy.out
            rz = rz_pool.tile([64, 512], F16, tag="rz",
                              name=f"rz{lq}_{ec}_{hh}")
            with nc.allow_low_precision(reason="1/Z in f16"):
                nc.vector.reciprocal(rz, avc[hh][64:128, :])
            nc.vector.tensor_mul(
                out=attnT[hh * 64:hh * 64 + 64, ec, :],
                in0=avc[hh][0:64, :], in1=rz)
        return f

    def slot_outproj(attnT, lq, i):
        def f():
            lt = lq * 4 + i
            op = aux_psum(f"op_{lq}_{i}")
            for g2 in range(2):
                mm(op, lhsT=attnT[:, g2, i * 128:(i + 1) * 128],
                   rhs=wo_sb[:, g2, :], start=(g2 == 0), stop=(g2 == 1))
            ob = ob_pool.tile([128, D], F16, tag="ob", name=f"ob_{lq}_{i}")
            nc.vector.tensor_copy(out=ob, in_=op)
            nc.sync.dma_start(out=io["out"][lt * 128:(lt + 1) * 128, :], in_=ob)
        return f

    def slot_qproj(lq, ec):
        def f():
            proj_qk_ec(state["xq_next"], wq_sb, bq_sb, qT, lq + 1, ec,
                       aux_psum(f"ppq{lq + 1}_{ec}"))
            if ec == 1 and lq + 2 < LQ:
                state["xq_next"] = load_xt(xq_pool, io["xq"], lq + 2,
                                           f"xtq{lq + 2}")
        return f

    for lq in range(LQ):
        l0 = lq * 512
        attnT = at_pool.tile([128, 2, 512], F16, tag="at", name=f"at{lq}")
        for ec in range(2):
            slots, pending = pending, []
            av = None
            groups = GROUPS_EVEN if (lq * 2 + ec) % 2 == 0 else GROUPS_ODD
            for gi, (u0, n, a_side) in enumerate(groups):
                pool = ps_scA if a_side else ps_scB
                full = pool.tile([128, 4 if a_side else 2, 512], F32,
                                 tag="sc", name=f"sc{lq}_{ec}_{gi}")
                sc = full[:, 0:n, :]
                for k in range(n):
                    j, hh = (u0 + k) // 2, (u0 + k) % 2
                    o = hh * 64
                    mm(sc[:, k, :],
                       lhsT=kT[o:o + 64, ec, j * 128:(j + 1) * 128],
                       rhs=qT[o:o + 64, ec, l0:l0 + 512],
                       start=True, stop=True, tile_position=(o, 0))
                ep_full = e_pool.tile([128, 4, 512], F16, tag="ep",
                                      name=f"ep{lq}_{ec}_{gi}")
                ep = ep_full[:, 0:n, :]
                nc.scalar.activation(out=ep, in_=sc,
                                     func=mybir.ActivationFunctionType.Exp,
                                     bias=shift_col[:, 0:1], scale=1.0)
                if gi < len(slots):
                    slots[gi]()
                if gi == 0:
                    av = [ps_av.tile([128, 512], F32, tag="av",
                                     name=f"av{lq}_{ec}_{i}")
                          for i in range(2)]
                for k in range(n):
                    j, hh = (u0 + k) // 2, (u0 + k) % 2
                    mm(av[hh], lhsT=v_sb[:, j, 2 * ec + hh, :],
                       rhs=ep[:, k, :], start=(j == 0), stop=(j == ST - 1))
            pending.append(slot_avcopy(av, lq, ec))
            pending.append(slot_norm(attnT, lq, ec, 0))
            pending.append(slot_norm(attnT, lq, ec, 1))
            if ec == 0:
                if lq + 1 < LQ:
                    pending.append(slot_qproj(lq, 0))
                    pending.append(slot_qproj(lq, 1))
            else:
                for i in range(4):
                    pending.append(slot_outproj(attnT, lq, i))
    for f in pending:
        f()


def build_nc():
    nc = bacc.Bacc()
    io = {}
    io["xq"] = nc.declare_dram_parameter("xq", [128, DC, L], F16, isOutput=False)
    io["xk"] = nc.declare_dram_parameter("xk", [128, DC, S], F16, isOutput=False)
    io["xv"] = nc.declare_dram_parameter("xv", [128, DC, S], F16, isOutput=False)
    io["wq"] = nc.declare_dram_parameter("wq", [128, DC, EG], F16, isOutput=False)
    io["wk"] = nc.declare_dram_parameter("wk", [128, DC, EG], F16, isOutput=False)
    io["wv"] = nc.declare_dram_parameter("wv", [128, DC, EG], F16, isOutput=False)
    io["wo"] = nc.declare_dram_parameter("wo", [128, 2, D], F16, isOutput=False)
    io["bq"] = nc.declare_dram_parameter("bq", [128, 2], F32, isOutput=False)
    io["bk"] = nc.declare_dram_parameter("bk", [128, 2], F32, isOutput=False)
    io["out"] = nc.declare_dram_parameter("out", [L, D], F16, isOutput=True)
    with tile.TileContext(nc) as tc:
        with ExitStack() as ctx:
            _emit(ctx, tc, io)
    nc.compile()
    return nc


_NC = None


def _get_nc():
    global _NC
    if _NC is None:
        _NC = build_nc()
    return _NC


def _chunk_w(w):
    n = w.shape[1]
    return np.ascontiguousarray(
        w.reshape(DC, 128, n).transpose(1, 0, 2), dtype=np.float16)


def _chunk_xt(x):
    return np.ascontiguousarray(
        np.asarray(x, dtype=np.float16).T.reshape(DC, 128, L).transpose(1, 0, 2))


def make_in_maps(queries, keys, values, tau, Wq, bq, Wk, bk, Wv, bv, Wo):
    xts = [{"xq": _chunk_xt(queries[b]), "xk": _chunk_xt(keys[b]),
            "xv": _chunk_xt(values[b])} for b in range(B)]
    in_maps = []
    for c in range(N_CORES):
        b, g = c // 2, c % 2
        e0 = g * EG
        f = np.float32(SCALE * tau[b])
        in_maps.append({
            **xts[b],
            "wq": _chunk_w(Wq[:, e0:e0 + EG] * f),
            "wk": _chunk_w(Wk[:, e0:e0 + EG]),
            "wv": _chunk_w(Wv[:, e0:e0 + EG]),
            "wo": np.ascontiguousarray(
                Wo[e0:e0 + EG, :].reshape(2, 128, D).transpose(1, 0, 2),
                dtype=np.float16),
            "bq": np.ascontiguousarray(
                (bq[e0:e0 + EG] * f).reshape(2, 128).T, dtype=np.float32),
            "bk": np.ascontiguousarray(
                bk[e0:e0 + EG].reshape(2, 128).T, dtype=np.float32),
        })
    return in_maps


def kernel(queries, keys, values, tau, delta, Wq, bq, Wk, bk, Wv, bv, Wo, bo,
           **_unused):
    queries = np.asarray(queries, dtype=np.float32)
    keys = np.asarray(keys, dtype=np.float32)
    values = np.asarray(values, dtype=np.float32)
    tau = np.asarray(tau, dtype=np.float32)
    Wq, bq = np.asarray(Wq, np.float32), np.asarray(bq, np.float32)
    Wk, bk = np.asarray(Wk, np.float32), np.asarray(bk, np.float32)
    Wv, bv = np.asarray(Wv, np.float32), np.asarray(bv, np.float32)
    Wo, bo = np.asarray(Wo, np.float32), np.asarray(bo, np.float32)

    nc = _get_nc()
    in_maps = make_in_maps(queries, keys, values, tau, Wq, bq, Wk, bk, Wv, bv, Wo)
    res = run_bass_kernel_spmd(nc, in_maps, list(range(N_CORES)))
    const_row = (bv @ Wo + bo).astype(np.float32)  # [512]
    out = np.empty((B, L, D), dtype=np.float32)
    for b in range(B):
        out[b] = res.results[2 * b]["out"].astype(np.float32) \
            + res.results[2 * b + 1]["out"].astype(np.float32) + const_row
    return out


if __name__ == "__main__":
    nc = build_nc()
    print("built OK")
